# revision 18
# baseline (speedup 1.0000x reference)
"""Distributed brute-force KNN (IndexFlatL2, K=3) + mean of gathered pred values.

Strategy (data-parallel over the memory bank N, queries replicated):
  - Host sorts the memory rows by ||m||^2 and shards the sorted bank across
    the 8 cores (12500 rows each), packed group-contiguously so each DMA is
    one long per-partition run.
  - Device: c[b, n] = (2q).m_n via fp8e4m3 DoubleRow matmuls (contraction
    pairs of k-subtiles) into fp32 PSUM; DVE tensor_reduce window-maxes each
    group's PSUM (windows of 10 rows) and the raw window maxes are DMA'd to
    DRAM. That's ALL the device does — no on-device selection, so the DVE
    work (~110us) hides entirely under the PE matmul stream (~170us).
  - Host: corrects window maxes by the window-mean ||m||^2 (rows are
    msq-sorted, so ||m||^2 is ~constant within a window), takes the global
    top-WSEL windows per query over all 8*1250 windows, exactly re-scores
    their rows (fp64), takes the true top-3, gathers pred_values, returns
    the mean.
"""

import sys
import types

import ml_dtypes
import numpy as np

try:  # bass_utils' axon trace path imports this unconditionally when
    import antenv.axon_hooks  # noqa: F401  # BASS_TRACE is set; stub it if absent
except ImportError:
    _stub = types.ModuleType("antenv.axon_hooks")
    _stub._hook = None

    def _set_hook(hook):
        _stub._hook = hook

    _stub.get_axon_ntff_profile_hook = lambda: _stub._hook
    _stub.set_axon_ntff_profile_hook = _set_hook
    sys.modules["antenv.axon_hooks"] = _stub
    try:  # the boot path degrades silently when antenv.axon_hooks is
        # missing; re-run its ctypes NTFF hook registration against our stub
        from trn_agent_boot.trn_boot import _ntff_profile_via_ctypes

        _hook = _ntff_profile_via_ctypes("/opt/axon/libaxon_pjrt.so")
        if _hook is not None:
            _stub._hook = _hook
    except Exception:
        pass

import concourse.bacc as bacc
import concourse.mybir as mybir
import concourse.tile as tile
from concourse import bass_utils

B = 1024            # queries
D = 1024            # embedding dim
N = 100000          # memory rows
NCORES = 8
NS = N // NCORES    # 12500 memory rows per core
BLK = 500           # matmul free-dim tile (fits one PSUM bank in fp32)
BLKP = 512          # padded block pitch in PSUM (one full bank)
NBLK = NS // BLK    # 25 blocks per core
GROUP_W = 2         # blocks per PSUM group (2 banks; ring of 4 = 8 banks)
KT = D // 128       # 8 contraction tiles
BCH = B // 128      # 8 query chunks of 128
WND = 10            # window width for the DVE windowed max
NWIN = NS // WND    # 1250 windows per core
WPB = BLK // WND    # 50 windows per block
K = 3
WSEL = 32           # windows exactly re-scored on host per query

# group widths: small first group so the first matmuls start early, small
# last group so the final reduce+DMA tail is short
_WIDTHS = [1] + [2] * 12
assert sum(_WIDTHS) == NBLK
GROUPS = []         # (block0, nblocks)
_g0 = 0
for _w in _WIDTHS:
    GROUPS.append((_g0, _w))
    _g0 += _w
NWARM = 10          # scratch matmuls to lift the PE HAM clock-gate early

_CACHE = {}
LAST_RUN = None
LAST_TOP_IDX = None


def _build_program():
    nc = bacc.Bacc(
        "TRN2",
        target_bir_lowering=False,
        debug=False,
        enable_asserts=False,
        num_devices=NCORES,
    )
    f32 = mybir.dt.float32
    mmdt = mybir.dt.float8e4

    # Inputs, packed host-side for single-run-per-partition DMAs:
    #   mg: per partition p, groups concatenated; group g holds KT chunks of
    #       its columns, i.e. mg[p, g0*KT*BLK*w ... ] = mT[o*128+p, cols]
    #   qg: [128, BCH, KT, 128] — query chunk-major so chunk 0 lands first
    mg = nc.dram_tensor("mg", [128, NS * KT], mmdt, kind="ExternalInput").ap()
    qg = nc.dram_tensor("qg", [128, BCH, KT, 128], mmdt, kind="ExternalInput").ap()
    out_w = nc.dram_tensor("out_w", [B, NWIN], f32, kind="ExternalOutput").ap()
    ow_r = out_w.rearrange("(c p) j -> p c j", p=128)

    with tile.TileContext(nc) as tc:
        with (
            tc.tile_pool(name="const", bufs=1) as cpool,
            tc.tile_pool(name="mov", bufs=2) as movpool,
            tc.tile_pool(name="wm", bufs=2) as wmpool,
            tc.tile_pool(name="psum", bufs=4, space="PSUM") as pspool,
        ):
            # PE warm-up: the HAM clock gate holds the PE at 1.2 GHz until it
            # has been busy ~3.4us. Run scratch matmuls while the input DMAs
            # stream so the real stream starts at 2.4 GHz.
            warm_w = cpool.tile([128, 128], mmdt, tag="warm_w")
            warm_x = cpool.tile([128, BLKP], mmdt, tag="warm_x")
            nc.vector.memset(warm_w, 0)
            nc.vector.memset(warm_x, 0)
            ps_warm = pspool.tile([128, GROUP_W, BLKP], f32, tag="ps", name="warm")
            for _ in range(NWARM):
                nc.tensor.matmul(ps_warm[:, 0, :], lhsT=warm_w, rhs=warm_x)

            qt_sb = cpool.tile([128, BCH, KT, 128], mmdt, tag="qt")
            # chunk 0 first (tiny), then the first mov group, then the rest of
            # the chunks — so the first matmuls are unblocked early
            nc.sync.dma_start(qt_sb[:, :1], qg[:, :1])
            first = True
            for blk0, w in GROUPS:
                wn = w * BLK
                mov = movpool.tile([128, KT, GROUP_W * BLK], mmdt, tag="mov")
                src = mg[:, blk0 * KT * BLK : blk0 * KT * BLK + KT * wn]
                src_r = src.rearrange("p (o n) -> p o n", o=KT)
                nc.sync.dma_start(mov[:, :, :wn], src_r)
                if first:
                    for bc in range(1, BCH):
                        nc.sync.dma_start(
                            qt_sb[:, bc : bc + 1], qg[:, bc : bc + 1]
                        )
                    first = False
                wmax = wmpool.tile([128, BCH, GROUP_W * WPB], f32, tag="wmax")
                for bc in range(BCH):
                    ps = pspool.tile([128, GROUP_W, BLKP], f32, tag="ps", name="ps")
                    for k in range(0, KT, 2):
                        lhsT = qt_sb[:, bc, k : k + 2, :]
                        for j in range(w):
                            nc.tensor.matmul(
                                ps[:, j, :BLK],
                                lhsT=lhsT,
                                rhs=mov[:, k : k + 2, j * BLK : (j + 1) * BLK],
                                start=(k == 0),
                                stop=(k + 2 >= KT),
                                perf_mode=mybir.MatmulPerfMode.DoubleRow,
                            )
                    nc.vector.tensor_reduce(
                        wmax[:, bc, : w * WPB].rearrange("p (j w) -> p j w", j=w),
                        ps[:, :w, :BLK].rearrange("p j (w t) -> p j w t", t=WND),
                        axis=mybir.AxisListType.X,
                        op=mybir.AluOpType.max,
                        opt_input=False,
                    )
                if blk0 + w == NBLK:
                    # last group: per-chunk DMAs so the final transfer only
                    # trails the last reduce, not all eight
                    for bc in range(BCH):
                        nc.sync.dma_start(
                            ow_r[:, bc : bc + 1, blk0 * WPB : blk0 * WPB + w * WPB],
                            wmax[:, bc : bc + 1, : w * WPB],
                        )
                else:
                    nc.sync.dma_start(
                        ow_r[:, :, blk0 * WPB : blk0 * WPB + w * WPB],
                        wmax[:, :, : w * WPB],
                    )
    nc.compile()
    return nc


def _pack_inputs(q, m_s):
    """Pack fp8 operands into the DMA-friendly layouts _build_program expects."""
    fp8 = ml_dtypes.float8_e4m3
    qTs = (np.ascontiguousarray(q.T) * np.float32(2.0)).astype(fp8)  # [D, B]
    # qg[p, c, o, i] = 2*q[c*128+i, o*128+p]
    qg = np.ascontiguousarray(
        qTs.reshape(KT, 128, BCH, 128).transpose(1, 2, 0, 3)
    )
    mgs = []
    for c in range(NCORES):
        mTc = np.ascontiguousarray(m_s[c * NS : (c + 1) * NS].T).astype(fp8)
        a = mTc.reshape(KT, 128, NS)  # [o, p, n]
        parts = [
            a[:, :, b0 * BLK : (b0 + w) * BLK].transpose(1, 0, 2).reshape(128, -1)
            for b0, w in GROUPS
        ]
        mgs.append(np.ascontiguousarray(np.concatenate(parts, axis=1)))
    return qg, mgs


def kernel(h_query, memory_embeds, pred_values):
    global LAST_RUN, LAST_TOP_IDX
    q = np.ascontiguousarray(np.asarray(h_query, dtype=np.float32))
    m = np.ascontiguousarray(np.asarray(memory_embeds, dtype=np.float32))
    pv = np.asarray(pred_values, dtype=np.float32)

    msq_full = np.einsum("nd,nd->n", m, m)
    perm = np.argsort(msq_full, kind="stable")
    m_s = m[perm]                      # msq-sorted memory bank
    msq_s = msq_full[perm]
    msqw_all = msq_s.reshape(N // WND, WND).mean(axis=1).astype(np.float32)

    qg, mgs = _pack_inputs(q, m_s)

    if "nc" not in _CACHE:
        _CACHE["nc"] = _build_program()
    nc = _CACHE["nc"]

    in_maps = [{"mg": mgs[c], "qg": qg} for c in range(NCORES)]
    res = bass_utils.run_bass_kernel_spmd(nc, in_maps, core_ids=list(range(NCORES)))
    LAST_RUN = res

    # [B, 8*NWIN] raw window maxes of 2q.m; window w covers sorted rows
    # [w*WND, +WND). Correct by window-mean ||m||^2 and pick global top-WSEL.
    wmax = np.concatenate([r["out_w"] for r in res.results], axis=1)
    wsc = wmax - msqw_all[None, :]
    sel = np.argpartition(-wsc, WSEL, axis=1)[:, :WSEL]    # [B, WSEL] windows
    rows = sel[:, :, None] * WND + np.arange(WND)[None, None, :]
    cidx = rows.reshape(B, WSEL * WND)                     # sorted-space rows
    mg_rows = m_s[cidx].astype(np.float64)                 # [B, WSEL*WND, D]
    s_exact = 2.0 * np.einsum("bd,bkd->bk", q.astype(np.float64), mg_rows)
    s_exact -= np.einsum("bkd,bkd->bk", mg_rows, mg_rows)
    pick = np.argpartition(-s_exact, K, axis=1)[:, :K]
    top_sorted = np.take_along_axis(cidx, pick, axis=1)
    top_idx = perm[top_sorted]                             # original row ids
    LAST_TOP_IDX = top_idx
    y = pv[top_idx].astype(np.float64).mean()
    return np.float32(y)
